# revision 18
# baseline (speedup 1.0000x reference)
"""ALiBi multi-head causal attention on 8 TRN2 NeuronCores.

Sharding: each core owns ONE batch (b = core//4) and FOUR heads, one from
each "band group". ALiBi weights decay as e^(-slope*dist), so head h only
needs keys within dist <= d_h = 34/slope_h of the query (dropped tail is
< e^-22 relative — far below the error budget). Heads are sorted by band
width and grouped in fours; group s's band (max over the group) is baked
into the one SPMD graph as slot s, and core c takes member c%4 of each
group. This balances the sparse-attention work across cores while keeping
a single NEFF. Bands are computed from the actual slopes in the input, so
a non-decaying alibi (e.g. zeros) degrades to full causal attention, never
to a wrong answer.

Device algorithm per head slot:
  - Q^T, K^T feature-on-partition from the projections; one extra
    contraction row carries -slope*q (bf16 — any per-q shift cancels in
    softmax, it only needs to keep exp in range), so the scores matmul
    S^T[k, q] lands pre-shifted.
  - exp(S^T + slope*k - 8) on ScalarE with per-partition f32 bias; softmax
    over k (the partition axis) needs no reduction: the denominator is the
    ones-column of V through the same PV matmul.
  - Causality: future tiles are never emitted; diagonal tiles get a
    triangular affine_select zero-fill after exp. Band: tiles left of
    q - d_slot are never emitted.
  - PV accumulates O^T (+denominator row) in PSUM over k-chunks; normalize
    via reciprocal + DRAM-bounce partition broadcast + multiply.
  - Output projection accumulates both 128-feature halves into [t, d] and
    streams out. Host sums the 4 per-core partials of each batch, adds bo.
"""

import sys

sys.path.insert(0, "/opt/trn_rl_repo")

import math

import numpy as np

B, T, D, H = 2, 2048, 1024, 16
DK = 64
NCORES = 8
HPS = 4  # head slots per core
FS = HPS * DK  # feature slice per core = 256

TQ = 512  # q-group width (one fp32 psum bank)
TK = 128  # k-chunk width (partition dim)
DC = 128  # projection contraction chunk
MARGIN = 34.0  # band cut: contributions with slope*dist >= MARGIN dropped

_NC_CACHE = {}


def build_nc(bands, t_sz=T, d_sz=D):
    import concourse.bass as bass
    import concourse.mybir as mybir
    import concourse.tile as tile
    from concourse import bacc

    fp32 = mybir.dt.float32
    bf16 = mybir.dt.bfloat16
    EXP = mybir.ActivationFunctionType.Exp

    n_dc = d_sz // DC
    n_kc = t_sz // TK
    n_qg = t_sz // TQ
    n_tc = t_sz // TK

    nc = bacc.Bacc("TRN2", target_bir_lowering=False, debug=False)

    qT = nc.declare_dram_parameter("qT", [d_sz, t_sz], bf16, isOutput=False)
    kT = nc.declare_dram_parameter("kT", [d_sz, t_sz], bf16, isOutput=False)
    vT = nc.declare_dram_parameter("vT", [d_sz, t_sz], bf16, isOutput=False)
    wq = nc.declare_dram_parameter("wq", [n_dc, DC, FS], bf16, isOutput=False)
    wk = nc.declare_dram_parameter("wk", [n_dc, DC, FS], bf16, isOutput=False)
    wv = nc.declare_dram_parameter("wv", [n_dc, DC, FS], bf16, isOutput=False)
    wo = nc.declare_dram_parameter("wo", [2, DC, d_sz], bf16, isOutput=False)
    qaug = nc.declare_dram_parameter("qaug", [HPS, t_sz], bf16, isOutput=False)
    ident = nc.declare_dram_parameter("ident", [128, 128], bf16, isOutput=False)
    ebias = nc.declare_dram_parameter("ebias", [TK, HPS, n_kc], fp32, isOutput=False)
    out = nc.declare_dram_parameter("out", [t_sz, d_sz], fp32, isOutput=True)

    # tiling plan per (slot, q-half): (j, g, s0, start, stop) per score tile.
    # q-halves let the [65, t_sz/2] O^T accumulator double-buffer in PSUM so
    # the next half's PV work overlaps this half's normalization tail.
    n_sp = 2 if n_qg >= 2 else 1  # q-half splits
    n_gh = n_qg // n_sp  # q-groups per half
    plans = []
    for s in range(HPS):
        d = int(bands[s])
        half_plans = []
        for qh in range(n_sp):
            tiles = []
            first_j = {}
            last_j = {}
            for j in range(n_kc):
                g0 = (j * TK) // TQ
                gmax = min(n_qg - 1, (j * TK + TK - 1 + d) // TQ)
                for g in range(max(g0, qh * n_gh), min(gmax, (qh + 1) * n_gh - 1) + 1):
                    if g not in first_j:
                        first_j[g] = j
                    last_j[g] = j
                    tiles.append((j, g))
            plan = []
            for j, g in tiles:
                s0 = j * TK - g * TQ if j * TK > g * TQ else 0
                plan.append((j, g, s0, j == first_j[g], j == last_j[g]))
            half_plans.append(plan)
        plans.append(half_plans)

    with tile.TileContext(nc) as tc:
        with (
            tc.tile_pool(name="const", bufs=1) as cpool,
            tc.tile_pool(name="proj", bufs=2) as projpool,
            tc.tile_pool(name="inp", bufs=8) as inpool,
            tc.tile_pool(name="work", bufs=4) as wpool,
            tc.tile_pool(name="ps", bufs=2, space="PSUM") as ps,
            tc.tile_pool(name="otps", bufs=1, space="PSUM") as otps,
            tc.tile_pool(name="dram", bufs=2, space="DRAM") as dpool,
        ):
            # ---- constants ----
            wq_sb = cpool.tile([DC, n_dc, FS], bf16)
            wk_sb = cpool.tile([DC, n_dc, FS], bf16)
            wv_sb = cpool.tile([DC, n_dc, FS], bf16)
            for w_sb, w_dr in ((wq_sb, wq), (wk_sb, wk), (wv_sb, wv)):
                nc.sync.dma_start(w_sb[:], w_dr.rearrange("c p f -> p c f"))
            wo_sb = cpool.tile([DC, 2, d_sz], bf16)
            nc.sync.dma_start(wo_sb[:], wo.rearrange("h p f -> p h f"))
            ebias_sb = cpool.tile([TK, HPS * n_kc], fp32)
            nc.sync.dma_start(ebias_sb[:], ebias.rearrange("p h j -> p (h j)"))
            ident_sb = cpool.tile([128, 128], bf16)
            nc.sync.dma_start(ident_sb[:], ident[:])

            # ---- phase A: projections -> per-slot QT/KT, per-fh VT ----
            qt_t = [
                projpool.tile([DK + 1, t_sz], bf16, tag="qt", name=f"qt{s}", bufs=HPS)
                for s in range(HPS)
            ]
            kt_t = [
                projpool.tile([DK + 1, t_sz], bf16, tag="kt", name=f"kt{s}", bufs=HPS)
                for s in range(HPS)
            ]
            vt_t = [
                projpool.tile([128, t_sz], bf16, tag="vt", name=f"vt{fh}", bufs=2)
                for fh in range(2)
            ]

            for s in range(HPS):
                nc.gpsimd.dma_start(qt_t[s][DK : DK + 1, :], qaug[s : s + 1, :])
                nc.gpsimd.memset(kt_t[s][DK : DK + 1, :], 1.0)

            for w_sb, xdr, kind in ((wq_sb, qT, "q"), (wk_sb, kT, "k"), (wv_sb, vT, "v")):
                for g in range(n_qg):
                    mm_f = [
                        ps.tile([128, TQ], fp32, tag="mm", name=f"mm{kind}{g}_{fh}")
                        for fh in range(2)
                    ]
                    for dc in range(n_dc):
                        xin = inpool.tile([DC, TQ], bf16, tag="xin")
                        nc.sync.dma_start(
                            xin[:],
                            xdr[dc * DC : (dc + 1) * DC, g * TQ : (g + 1) * TQ],
                        )
                        for fh in range(2):
                            nc.tensor.matmul(
                                mm_f[fh][:],
                                w_sb[:, dc, fh * DC : (fh + 1) * DC],
                                xin[:],
                                start=(dc == 0),
                                stop=(dc == n_dc - 1),
                            )
                    gs = slice(g * TQ, (g + 1) * TQ)
                    for fh in range(2):
                        if kind == "q":
                            nc.vector.tensor_copy(qt_t[2 * fh][0:DK, gs], mm_f[fh][0:DK, :])
                            nc.vector.tensor_copy(qt_t[2 * fh + 1][0:DK, gs], mm_f[fh][DK:128, :])
                        elif kind == "k":
                            nc.vector.tensor_copy(kt_t[2 * fh][0:DK, gs], mm_f[fh][0:DK, :])
                            nc.vector.tensor_copy(kt_t[2 * fh + 1][0:DK, gs], mm_f[fh][DK:128, :])
                        else:
                            nc.vector.tensor_copy(vt_t[fh][:, gs], mm_f[fh][:])

            # ---- phase A2: V transpose (PE) -> per-slot V_aug [k, j, DK+1] ----
            va_t = [
                projpool.tile([TK, n_kc, DK + 1], bf16, tag="va", name=f"va{s}", bufs=HPS)
                for s in range(HPS)
            ]
            for s in range(HPS):
                nc.gpsimd.memset(va_t[s][:, :, DK : DK + 1], 1.0)
            for fh in range(2):
                for j in range(n_kc):
                    vtr = ps.tile([128, 128], bf16, tag="st", name=f"vtr{fh}_{j}")
                    nc.tensor.transpose(
                        vtr[:], vt_t[fh][:, j * TK : (j + 1) * TK], ident_sb[:]
                    )
                    nc.vector.tensor_copy(va_t[2 * fh][:, j, 0:DK], vtr[:, 0:DK])
                    nc.vector.tensor_copy(va_t[2 * fh + 1][:, j, 0:DK], vtr[:, DK:128])

            # ---- phase B: attention per slot ----
            otn_t = [
                projpool.tile([128, t_sz], bf16, tag="otn", name=f"otn{fh}", bufs=2)
                for fh in range(2)
            ]
            for s in range(HPS):
              for qh in range(n_sp):
                hbase = qh * (t_sz // n_sp)
                ot = otps.tile([DK + 1, t_sz // n_sp], fp32, tag="ot", name=f"ot{s}_{qh}", bufs=2)
                by_j = {}
                for j, g, s0, first, last in plans[s][qh]:
                    by_j.setdefault(j, []).append((g, s0, first, last))
                for j, tiles_j in by_j.items():
                    pts = []
                    for g, s0, first, last in tiles_j:
                        st = ps.tile([128, TQ], fp32, tag="st", name=f"st{s}_{j}_{g}")
                        nc.tensor.matmul(
                            st[:, s0:TQ],
                            kt_t[s][:, j * TK : (j + 1) * TK],
                            qt_t[s][:, g * TQ + s0 : (g + 1) * TQ],
                            start=True,
                            stop=True,
                        )
                        pt = wpool.tile([128, TQ], bf16, tag="pt", name=f"pt{s}_{j}_{g}")
                        nc.scalar.activation(
                            pt[:, s0:TQ],
                            st[:, s0:TQ],
                            EXP,
                            bias=ebias_sb[:, s * n_kc + j : s * n_kc + j + 1],
                            scale=1.0,
                        )
                        if j * TK >= g * TQ:
                            nc.gpsimd.affine_select(
                                out=pt[:, s0 : s0 + TK],
                                in_=pt[:, s0 : s0 + TK],
                                compare_op=mybir.AluOpType.is_ge,
                                fill=0.0,
                                base=0,
                                pattern=[[1, TK]],
                                channel_multiplier=-1,
                            )
                        pts.append(pt)
                    for (g, s0, first, last), pt in zip(tiles_j, pts):
                        nc.tensor.matmul(
                            ot[:, g * TQ + s0 - hbase : (g + 1) * TQ - hbase],
                            va_t[s][:, j, :],
                            pt[:, s0:TQ],
                            start=first,
                            stop=last,
                        )
                # normalize: recip of denominator row, broadcast, multiply
                th = t_sz // n_sp
                recip = wpool.tile([1, th], fp32, tag="recip", bufs=2, name=f"recip{s}_{qh}")
                nc.vector.reciprocal(recip[0:1, :], ot[DK : DK + 1, :])
                rb_dram = dpool.tile([1, th], fp32, name=f"rbd{s}_{qh}")
                nc.sync.dma_start(rb_dram[:], recip[0:1, :])
                rb_ap = rb_dram[:]
                rb_bcast = type(rb_ap)(rb_ap.tensor, rb_ap.offset, [[0, DK], [1, th]])
                rbc = wpool.tile([DK, th], fp32, tag="rbc", bufs=2, name=f"rbc{s}_{qh}")
                nc.sync.dma_start(rbc[:], rb_bcast)
                nc.vector.tensor_tensor(
                    out=otn_t[s // 2][(s % 2) * DK : (s % 2) * DK + DK, hbase : hbase + th],
                    in0=ot[0:DK, :],
                    in1=rbc[:],
                    op=mybir.AluOpType.mult,
                )

            # ---- phase C: output projection ----
            n_dh = d_sz // TQ
            for tci in range(n_tc):
                ogs = [
                    ps.tile([128, TQ], fp32, tag="mm", name=f"og{tci}_{dh}")
                    for dh in range(n_dh)
                ]
                for fh in range(2):
                    for dh in range(n_dh):
                        nc.tensor.matmul(
                            ogs[dh][:],
                            otn_t[fh][:, tci * TK : (tci + 1) * TK],
                            wo_sb[:, fh, dh * TQ : (dh + 1) * TQ],
                            start=(fh == 0),
                            stop=(fh == 1),
                        )
                for dh in range(n_dh):
                    ob = wpool.tile([128, TQ], fp32, tag="ob", name=f"ob{tci}_{dh}")
                    if dh % 2 == 0:
                        nc.vector.tensor_copy(ob[:], ogs[dh][:])
                    else:
                        nc.scalar.copy(ob[:], ogs[dh][:])
                    nc.sync.dma_start(
                        out[tci * TK : (tci + 1) * TK, dh * TQ : (dh + 1) * TQ],
                        ob[:],
                    )

    nc.compile()
    return nc


def _plan_shards(alibi_bias, t_sz):
    """Head->slot assignment and per-slot bands from the actual slopes."""
    slopes = (-alibi_bias[:, 1, 0]).astype(np.float64)
    d = np.where(slopes > 0, np.ceil(MARGIN / np.maximum(slopes, 1e-30)), t_sz)
    d = np.minimum(d, t_sz).astype(np.int64)
    order = np.argsort(-d, kind="stable")  # widest band first
    groups = [order[4 * s : 4 * s + 4] for s in range(HPS)]
    bands = tuple(int(d[g].max()) for g in groups)
    core_heads = [[int(groups[s][c % 4]) for s in range(HPS)] for c in range(NCORES)]
    return bands, core_heads


def _host_prep(query, key, value, alibi_bias, Wq, Wk, Wv, Wo, core_heads):
    import ml_dtypes

    bf16 = ml_dtypes.bfloat16
    b_sz, t_sz, d_sz = query.shape

    slopes = (-alibi_bias[:, 1, 0]).astype(np.float32)

    identity = np.eye(128).astype(bf16)
    qTh = [np.ascontiguousarray(query[b].T).astype(bf16) for b in range(b_sz)]
    kTh = [np.ascontiguousarray(key[b].T).astype(bf16) for b in range(b_sz)]
    vTh = [np.ascontiguousarray(value[b].T).astype(bf16) for b in range(b_sz)]

    scale = 1.0 / math.sqrt(DK)
    n_dc = d_sz // DC
    n_kc = t_sz // TK
    qpos = np.arange(t_sz, dtype=np.float32)
    p = np.arange(TK, dtype=np.float32)
    jj = np.arange(n_kc, dtype=np.float32)
    kpos = jj[None, :] * TK + p[:, None]  # [TK, n_kc]

    per_b = NCORES // b_sz
    in_maps = []
    for c in range(NCORES):
        b = c // per_b
        heads = core_heads[c]
        rows = np.concatenate([np.arange(h * DK, (h + 1) * DK) for h in heads])
        hsl = slopes[heads]  # [HPS]
        wq_c = np.ascontiguousarray((Wq[rows] * scale).T.reshape(n_dc, DC, FS)).astype(bf16)
        wk_c = np.ascontiguousarray(Wk[rows].T.reshape(n_dc, DC, FS)).astype(bf16)
        wv_c = np.ascontiguousarray(Wv[rows].T.reshape(n_dc, DC, FS)).astype(bf16)
        wo_c = np.ascontiguousarray(Wo[:, rows].T.reshape(2, DC, d_sz)).astype(bf16)
        qaug_c = (-hsl[:, None] * qpos[None, :]).astype(bf16)
        ebias_c = np.ascontiguousarray(
            (hsl[None, :, None] * kpos[:, None, :] - 8.0).astype(np.float32)
        )
        in_maps.append(
            {
                "qT": qTh[b],
                "kT": kTh[b],
                "vT": vTh[b],
                "wq": wq_c,
                "wk": wk_c,
                "wv": wv_c,
                "wo": wo_c,
                "qaug": qaug_c,
                "ebias": ebias_c,
                "ident": identity,
            }
        )
    return in_maps


def _alibi_is_structured(alibi_bias):
    """Check bias[h,i,j] == slope_h*(j-i) on a sample grid."""
    hgrid = np.arange(alibi_bias.shape[0])
    igrid = np.linspace(0, alibi_bias.shape[1] - 1, 37).astype(np.int64)
    jgrid = np.linspace(0, alibi_bias.shape[2] - 1, 41).astype(np.int64)
    slopes = (-alibi_bias[:, 1, 0]).astype(np.float32)
    sample = alibi_bias[np.ix_(hgrid, igrid, jgrid)].astype(np.float32)
    dist = jgrid[None, :].astype(np.float32) - igrid[:, None].astype(np.float32)
    ref = slopes[:, None, None] * dist[None]
    return np.allclose(sample, ref, rtol=1e-5, atol=1e-6)


def _reference_fallback(query, key, value, alibi_bias, Wq, Wk, Wv, Wo, bo):
    b_sz, t_sz, d_sz = query.shape
    n_heads = alibi_bias.shape[0]
    dk = d_sz // n_heads
    q64, k64, v64 = (x.astype(np.float64) for x in (query, key, value))
    Q = (q64 @ Wq.T.astype(np.float64)).reshape(b_sz, t_sz, n_heads, dk)
    K = (k64 @ Wk.T.astype(np.float64)).reshape(b_sz, t_sz, n_heads, dk)
    V = (v64 @ Wv.T.astype(np.float64)).reshape(b_sz, t_sz, n_heads, dk)
    out = np.zeros((b_sz, t_sz, d_sz), dtype=np.float64)
    causal = np.triu(np.ones((t_sz, t_sz), dtype=bool), 1)
    for b in range(b_sz):
        for h in range(n_heads):
            s = (Q[b, :, h] @ K[b, :, h].T) / math.sqrt(dk) + alibi_bias[h]
            s = np.where(causal, -np.inf, s)
            s -= s.max(axis=-1, keepdims=True)
            pr = np.exp(s)
            pr /= pr.sum(axis=-1, keepdims=True)
            out[b, :, h * dk : (h + 1) * dk] = pr @ V[b, :, h]
    return (
        (out.reshape(b_sz * t_sz, d_sz) @ Wo.T.astype(np.float64) + bo)
        .reshape(b_sz, t_sz, d_sz)
        .astype(np.float32)
    )


def kernel(query, key, value, alibi_bias, Wq, Wk, Wv, Wo, bo):
    query = np.asarray(query, dtype=np.float32)
    key = np.asarray(key, dtype=np.float32)
    value = np.asarray(value, dtype=np.float32)
    alibi_bias = np.asarray(alibi_bias, dtype=np.float32)
    Wq = np.asarray(Wq, dtype=np.float32)
    Wk = np.asarray(Wk, dtype=np.float32)
    Wv = np.asarray(Wv, dtype=np.float32)
    Wo = np.asarray(Wo, dtype=np.float32)
    bo = np.asarray(bo, dtype=np.float32)

    if not _alibi_is_structured(alibi_bias):
        return _reference_fallback(query, key, value, alibi_bias, Wq, Wk, Wv, Wo, bo)

    from concourse import bass_utils

    bands, core_heads = _plan_shards(alibi_bias, query.shape[1])
    if bands not in _NC_CACHE:
        _NC_CACHE[bands] = build_nc(bands)
    nc = _NC_CACHE[bands]

    in_maps = _host_prep(query, key, value, alibi_bias, Wq, Wk, Wv, Wo, core_heads)
    res = bass_utils.run_bass_kernel_spmd(nc, in_maps, core_ids=list(range(NCORES)))
    b_sz, t_sz, d_sz = query.shape
    per_b = NCORES // b_sz
    outp = np.zeros((b_sz, t_sz, d_sz), dtype=np.float64)
    for c in range(NCORES):
        outp[c // per_b] += res.results[c]["out"]
    return (outp + bo).astype(np.float32)


if __name__ == "__main__":
    pass


# revision 19
# speedup vs baseline: 1.0457x; 1.0457x over previous
"""ALiBi multi-head causal attention on 8 TRN2 NeuronCores.

Sharding: each core owns ONE batch (b = core//4) and FOUR heads, one from
each "band group". ALiBi weights decay as e^(-slope*dist), so head h only
needs keys within dist <= d_h = 34/slope_h of the query (dropped tail is
< e^-22 relative — far below the error budget). Heads are sorted by band
width and grouped in fours; group s's band (max over the group) is baked
into the one SPMD graph as slot s, and core c takes member c%4 of each
group. This balances the sparse-attention work across cores while keeping
a single NEFF. Bands are computed from the actual slopes in the input, so
a non-decaying alibi (e.g. zeros) degrades to full causal attention, never
to a wrong answer.

Device algorithm per head slot:
  - Q^T, K^T feature-on-partition from the projections; one extra
    contraction row carries -slope*q (bf16 — any per-q shift cancels in
    softmax, it only needs to keep exp in range), so the scores matmul
    S^T[k, q] lands pre-shifted.
  - exp(S^T + slope*k - 8) on ScalarE with per-partition f32 bias; softmax
    over k (the partition axis) needs no reduction: the denominator is the
    ones-column of V through the same PV matmul.
  - Causality: future tiles are never emitted; diagonal tiles get a
    triangular affine_select zero-fill after exp. Band: tiles left of
    q - d_slot are never emitted.
  - PV accumulates O^T (+denominator row) in PSUM over k-chunks; normalize
    via reciprocal + DRAM-bounce partition broadcast + multiply.
  - Output projection accumulates both 128-feature halves into [t, d] and
    streams out. Host sums the 4 per-core partials of each batch, adds bo.
"""

import sys

sys.path.insert(0, "/opt/trn_rl_repo")

import math

import numpy as np

B, T, D, H = 2, 2048, 1024, 16
DK = 64
NCORES = 8
HPS = 4  # head slots per core
FS = HPS * DK  # feature slice per core = 256

TQ = 512  # q-group width (one fp32 psum bank)
TK = 128  # k-chunk width (partition dim)
DC = 128  # projection contraction chunk
MARGIN = 34.0  # band cut: contributions with slope*dist >= MARGIN dropped

_NC_CACHE = {}


def build_nc(bands, t_sz=T, d_sz=D):
    import concourse.bass as bass
    import concourse.mybir as mybir
    import concourse.tile as tile
    from concourse import bacc

    fp32 = mybir.dt.float32
    bf16 = mybir.dt.bfloat16
    EXP = mybir.ActivationFunctionType.Exp

    n_dc = d_sz // DC
    n_kc = t_sz // TK
    n_qg = t_sz // TQ
    n_tc = t_sz // TK

    nc = bacc.Bacc("TRN2", target_bir_lowering=False, debug=False)

    qT = nc.declare_dram_parameter("qT", [d_sz, t_sz], bf16, isOutput=False)
    kT = nc.declare_dram_parameter("kT", [d_sz, t_sz], bf16, isOutput=False)
    vT = nc.declare_dram_parameter("vT", [d_sz, t_sz], bf16, isOutput=False)
    wq = nc.declare_dram_parameter("wq", [n_dc, DC, FS], bf16, isOutput=False)
    wk = nc.declare_dram_parameter("wk", [n_dc, DC, FS], bf16, isOutput=False)
    wv = nc.declare_dram_parameter("wv", [n_dc, DC, FS], bf16, isOutput=False)
    wo = nc.declare_dram_parameter("wo", [2, DC, d_sz], bf16, isOutput=False)
    qaug = nc.declare_dram_parameter("qaug", [HPS, t_sz], bf16, isOutput=False)
    ident = nc.declare_dram_parameter("ident", [128, 128], bf16, isOutput=False)
    ebias = nc.declare_dram_parameter("ebias", [TK, HPS, n_kc], fp32, isOutput=False)
    out = nc.declare_dram_parameter("out", [t_sz, d_sz], fp32, isOutput=True)

    # tiling plan per (slot, q-half): (j, g, s0, start, stop) per score tile.
    # q-halves let the [65, t_sz/2] O^T accumulator double-buffer in PSUM so
    # the next half's PV work overlaps this half's normalization tail.
    n_sp = 2 if n_qg >= 2 else 1  # q-half splits
    n_gh = n_qg // n_sp  # q-groups per half
    plans = []
    for s in range(HPS):
        d = int(bands[s])
        half_plans = []
        for qh in range(n_sp):
            tiles = []
            first_j = {}
            last_j = {}
            for j in range(n_kc):
                g0 = (j * TK) // TQ
                gmax = min(n_qg - 1, (j * TK + TK - 1 + d) // TQ)
                for g in range(max(g0, qh * n_gh), min(gmax, (qh + 1) * n_gh - 1) + 1):
                    if g not in first_j:
                        first_j[g] = j
                    last_j[g] = j
                    tiles.append((j, g))
            plan = []
            for j, g in tiles:
                s0 = j * TK - g * TQ if j * TK > g * TQ else 0
                plan.append((j, g, s0, j == first_j[g], j == last_j[g]))
            half_plans.append(plan)
        plans.append(half_plans)

    with tile.TileContext(nc) as tc:
        with (
            tc.tile_pool(name="const", bufs=1) as cpool,
            tc.tile_pool(name="proj", bufs=2) as projpool,
            tc.tile_pool(name="inp", bufs=8) as inpool,
            tc.tile_pool(name="work", bufs=4) as wpool,
            tc.tile_pool(name="ps", bufs=2, space="PSUM") as ps,
            tc.tile_pool(name="otps", bufs=1, space="PSUM") as otps,
            tc.tile_pool(name="dram", bufs=2, space="DRAM") as dpool,
        ):
            # ---- constants ----
            wq_sb = cpool.tile([DC, n_dc, FS], bf16)
            wk_sb = cpool.tile([DC, n_dc, FS], bf16)
            wv_sb = cpool.tile([DC, n_dc, FS], bf16)
            for w_sb, w_dr in ((wq_sb, wq), (wk_sb, wk), (wv_sb, wv)):
                nc.sync.dma_start(w_sb[:], w_dr.rearrange("c p f -> p c f"))
            wo_sb = cpool.tile([DC, 2, d_sz], bf16)
            nc.sync.dma_start(wo_sb[:], wo.rearrange("h p f -> p h f"))
            ebias_sb = cpool.tile([TK, HPS * n_kc], fp32)
            nc.sync.dma_start(ebias_sb[:], ebias.rearrange("p h j -> p (h j)"))
            ident_sb = cpool.tile([128, 128], bf16)
            nc.sync.dma_start(ident_sb[:], ident[:])

            # ---- phase A: projections -> per-slot QT/KT, per-fh VT ----
            qt_t = [
                projpool.tile([DK + 1, t_sz], bf16, tag="qt", name=f"qt{s}", bufs=HPS)
                for s in range(HPS)
            ]
            kt_t = [
                projpool.tile([DK + 1, t_sz], bf16, tag="kt", name=f"kt{s}", bufs=HPS)
                for s in range(HPS)
            ]
            vt_t = [
                projpool.tile([128, t_sz], bf16, tag="vt", name=f"vt{fh}", bufs=2)
                for fh in range(2)
            ]

            for s in range(HPS):
                nc.gpsimd.dma_start(qt_t[s][DK : DK + 1, :], qaug[s : s + 1, :])
                nc.gpsimd.memset(kt_t[s][DK : DK + 1, :], 1.0)

            for w_sb, xdr, kind in ((wq_sb, qT, "q"), (wk_sb, kT, "k"), (wv_sb, vT, "v")):
                for g in range(n_qg):
                    mm_f = [
                        ps.tile([128, TQ], fp32, tag="mm", name=f"mm{kind}{g}_{fh}")
                        for fh in range(2)
                    ]
                    for dc in range(n_dc):
                        xin = inpool.tile([DC, TQ], bf16, tag="xin")
                        nc.sync.dma_start(
                            xin[:],
                            xdr[dc * DC : (dc + 1) * DC, g * TQ : (g + 1) * TQ],
                        )
                        for fh in range(2):
                            nc.tensor.matmul(
                                mm_f[fh][:],
                                w_sb[:, dc, fh * DC : (fh + 1) * DC],
                                xin[:],
                                start=(dc == 0),
                                stop=(dc == n_dc - 1),
                            )
                    gs = slice(g * TQ, (g + 1) * TQ)
                    for fh in range(2):
                        if kind == "q":
                            nc.vector.tensor_copy(qt_t[2 * fh][0:DK, gs], mm_f[fh][0:DK, :])
                            nc.vector.tensor_copy(qt_t[2 * fh + 1][0:DK, gs], mm_f[fh][DK:128, :])
                        elif kind == "k":
                            nc.vector.tensor_copy(kt_t[2 * fh][0:DK, gs], mm_f[fh][0:DK, :])
                            nc.vector.tensor_copy(kt_t[2 * fh + 1][0:DK, gs], mm_f[fh][DK:128, :])
                        else:
                            nc.vector.tensor_copy(vt_t[fh][:, gs], mm_f[fh][:])

            # ---- phase A2: V transpose (PE) -> per-slot V_aug [k, j, DK+1] ----
            va_t = [
                projpool.tile([TK, n_kc, DK + 1], bf16, tag="va", name=f"va{s}", bufs=HPS)
                for s in range(HPS)
            ]
            for s in range(HPS):
                nc.gpsimd.memset(va_t[s][:, :, DK : DK + 1], 1.0)
            for fh in range(2):
                for j in range(n_kc):
                    vtr = ps.tile([128, 128], bf16, tag="st", name=f"vtr{fh}_{j}")
                    nc.tensor.transpose(
                        vtr[:], vt_t[fh][:, j * TK : (j + 1) * TK], ident_sb[:]
                    )
                    nc.vector.tensor_copy(va_t[2 * fh][:, j, 0:DK], vtr[:, 0:DK])
                    nc.vector.tensor_copy(va_t[2 * fh + 1][:, j, 0:DK], vtr[:, DK:128])

            # ---- phase B: attention per slot ----
            otn_t = [
                projpool.tile([128, t_sz], bf16, tag="otn", name=f"otn{fh}", bufs=2)
                for fh in range(2)
            ]
            for s in range(HPS):
              for qh in range(n_sp):
                hbase = qh * (t_sz // n_sp)
                ot = otps.tile([DK + 1, t_sz // n_sp], fp32, tag="ot", name=f"ot{s}_{qh}", bufs=2)
                by_j = {}
                for j, g, s0, first, last in plans[s][qh]:
                    by_j.setdefault(j, []).append((g, s0, first, last))
                for j, tiles_j in by_j.items():
                    pts = []
                    for g, s0, first, last in tiles_j:
                        st = ps.tile([128, TQ], fp32, tag="st", name=f"st{s}_{j}_{g}")
                        nc.tensor.matmul(
                            st[:, s0:TQ],
                            kt_t[s][:, j * TK : (j + 1) * TK],
                            qt_t[s][:, g * TQ + s0 : (g + 1) * TQ],
                            start=True,
                            stop=True,
                        )
                        pt = wpool.tile([128, TQ], bf16, tag="pt", name=f"pt{s}_{j}_{g}")
                        nc.scalar.activation(
                            pt[:, s0:TQ],
                            st[:, s0:TQ],
                            EXP,
                            bias=ebias_sb[:, s * n_kc + j : s * n_kc + j + 1],
                            scale=1.0,
                        )
                        if j * TK >= g * TQ:
                            nc.gpsimd.affine_select(
                                out=pt[:, s0 : s0 + TK],
                                in_=pt[:, s0 : s0 + TK],
                                compare_op=mybir.AluOpType.is_ge,
                                fill=0.0,
                                base=0,
                                pattern=[[1, TK]],
                                channel_multiplier=-1,
                            )
                        pts.append(pt)
                    for (g, s0, first, last), pt in zip(tiles_j, pts):
                        nc.tensor.matmul(
                            ot[:, g * TQ + s0 - hbase : (g + 1) * TQ - hbase],
                            va_t[s][:, j, :],
                            pt[:, s0:TQ],
                            start=first,
                            stop=last,
                        )
                # normalize: recip of denominator row, broadcast, multiply
                th = t_sz // n_sp
                recip = wpool.tile([1, th], fp32, tag="recip", bufs=2, name=f"recip{s}_{qh}")
                nc.vector.reciprocal(recip[0:1, :], ot[DK : DK + 1, :])
                rb_dram = dpool.tile([1, th], fp32, name=f"rbd{s}_{qh}")
                nc.sync.dma_start(rb_dram[:], recip[0:1, :])
                rb_ap = rb_dram[:]
                rb_bcast = type(rb_ap)(rb_ap.tensor, rb_ap.offset, [[0, DK], [1, th]])
                rbc = wpool.tile([DK, th], fp32, tag="rbc", bufs=2, name=f"rbc{s}_{qh}")
                nc.sync.dma_start(rbc[:], rb_bcast)
                nc.vector.tensor_tensor(
                    out=otn_t[s // 2][(s % 2) * DK : (s % 2) * DK + DK, hbase : hbase + th],
                    in0=ot[0:DK, :],
                    in1=rbc[:],
                    op=mybir.AluOpType.mult,
                )

            # ---- phase C: output projection ----
            for tci in range(n_tc):
                for dh in range(d_sz // TQ):
                    og = ps.tile([128, TQ], fp32, tag="mm", name=f"og{tci}_{dh}")
                    for fh in range(2):
                        nc.tensor.matmul(
                            og[:],
                            otn_t[fh][:, tci * TK : (tci + 1) * TK],
                            wo_sb[:, fh, dh * TQ : (dh + 1) * TQ],
                            start=(fh == 0),
                            stop=(fh == 1),
                        )
                    ob = wpool.tile([128, TQ], fp32, tag="ob", name=f"ob{tci}_{dh}")
                    if dh % 2 == 0:
                        nc.vector.tensor_copy(ob[:], og[:])
                    else:
                        nc.scalar.copy(ob[:], og[:])
                    nc.sync.dma_start(
                        out[tci * TK : (tci + 1) * TK, dh * TQ : (dh + 1) * TQ],
                        ob[:],
                    )

    nc.compile()
    return nc


def _plan_shards(alibi_bias, t_sz):
    """Head->slot assignment and per-slot bands from the actual slopes."""
    slopes = (-alibi_bias[:, 1, 0]).astype(np.float64)
    d = np.where(slopes > 0, np.ceil(MARGIN / np.maximum(slopes, 1e-30)), t_sz)
    d = np.minimum(d, t_sz).astype(np.int64)
    order = np.argsort(-d, kind="stable")  # widest band first
    groups = [order[4 * s : 4 * s + 4] for s in range(HPS)]
    bands = tuple(int(d[g].max()) for g in groups)
    core_heads = [[int(groups[s][c % 4]) for s in range(HPS)] for c in range(NCORES)]
    return bands, core_heads


def _host_prep(query, key, value, alibi_bias, Wq, Wk, Wv, Wo, core_heads):
    import ml_dtypes

    bf16 = ml_dtypes.bfloat16
    b_sz, t_sz, d_sz = query.shape

    slopes = (-alibi_bias[:, 1, 0]).astype(np.float32)

    identity = np.eye(128).astype(bf16)
    qTh = [np.ascontiguousarray(query[b].T).astype(bf16) for b in range(b_sz)]
    kTh = [np.ascontiguousarray(key[b].T).astype(bf16) for b in range(b_sz)]
    vTh = [np.ascontiguousarray(value[b].T).astype(bf16) for b in range(b_sz)]

    scale = 1.0 / math.sqrt(DK)
    n_dc = d_sz // DC
    n_kc = t_sz // TK
    qpos = np.arange(t_sz, dtype=np.float32)
    p = np.arange(TK, dtype=np.float32)
    jj = np.arange(n_kc, dtype=np.float32)
    kpos = jj[None, :] * TK + p[:, None]  # [TK, n_kc]

    per_b = NCORES // b_sz
    in_maps = []
    for c in range(NCORES):
        b = c // per_b
        heads = core_heads[c]
        rows = np.concatenate([np.arange(h * DK, (h + 1) * DK) for h in heads])
        hsl = slopes[heads]  # [HPS]
        wq_c = np.ascontiguousarray((Wq[rows] * scale).T.reshape(n_dc, DC, FS)).astype(bf16)
        wk_c = np.ascontiguousarray(Wk[rows].T.reshape(n_dc, DC, FS)).astype(bf16)
        wv_c = np.ascontiguousarray(Wv[rows].T.reshape(n_dc, DC, FS)).astype(bf16)
        wo_c = np.ascontiguousarray(Wo[:, rows].T.reshape(2, DC, d_sz)).astype(bf16)
        qaug_c = (-hsl[:, None] * qpos[None, :]).astype(bf16)
        ebias_c = np.ascontiguousarray(
            (hsl[None, :, None] * kpos[:, None, :] - 8.0).astype(np.float32)
        )
        in_maps.append(
            {
                "qT": qTh[b],
                "kT": kTh[b],
                "vT": vTh[b],
                "wq": wq_c,
                "wk": wk_c,
                "wv": wv_c,
                "wo": wo_c,
                "qaug": qaug_c,
                "ebias": ebias_c,
                "ident": identity,
            }
        )
    return in_maps


def _alibi_is_structured(alibi_bias):
    """Check bias[h,i,j] == slope_h*(j-i) on a sample grid."""
    hgrid = np.arange(alibi_bias.shape[0])
    igrid = np.linspace(0, alibi_bias.shape[1] - 1, 37).astype(np.int64)
    jgrid = np.linspace(0, alibi_bias.shape[2] - 1, 41).astype(np.int64)
    slopes = (-alibi_bias[:, 1, 0]).astype(np.float32)
    sample = alibi_bias[np.ix_(hgrid, igrid, jgrid)].astype(np.float32)
    dist = jgrid[None, :].astype(np.float32) - igrid[:, None].astype(np.float32)
    ref = slopes[:, None, None] * dist[None]
    return np.allclose(sample, ref, rtol=1e-5, atol=1e-6)


def _reference_fallback(query, key, value, alibi_bias, Wq, Wk, Wv, Wo, bo):
    b_sz, t_sz, d_sz = query.shape
    n_heads = alibi_bias.shape[0]
    dk = d_sz // n_heads
    q64, k64, v64 = (x.astype(np.float64) for x in (query, key, value))
    Q = (q64 @ Wq.T.astype(np.float64)).reshape(b_sz, t_sz, n_heads, dk)
    K = (k64 @ Wk.T.astype(np.float64)).reshape(b_sz, t_sz, n_heads, dk)
    V = (v64 @ Wv.T.astype(np.float64)).reshape(b_sz, t_sz, n_heads, dk)
    out = np.zeros((b_sz, t_sz, d_sz), dtype=np.float64)
    causal = np.triu(np.ones((t_sz, t_sz), dtype=bool), 1)
    for b in range(b_sz):
        for h in range(n_heads):
            s = (Q[b, :, h] @ K[b, :, h].T) / math.sqrt(dk) + alibi_bias[h]
            s = np.where(causal, -np.inf, s)
            s -= s.max(axis=-1, keepdims=True)
            pr = np.exp(s)
            pr /= pr.sum(axis=-1, keepdims=True)
            out[b, :, h * dk : (h + 1) * dk] = pr @ V[b, :, h]
    return (
        (out.reshape(b_sz * t_sz, d_sz) @ Wo.T.astype(np.float64) + bo)
        .reshape(b_sz, t_sz, d_sz)
        .astype(np.float32)
    )


def kernel(query, key, value, alibi_bias, Wq, Wk, Wv, Wo, bo):
    query = np.asarray(query, dtype=np.float32)
    key = np.asarray(key, dtype=np.float32)
    value = np.asarray(value, dtype=np.float32)
    alibi_bias = np.asarray(alibi_bias, dtype=np.float32)
    Wq = np.asarray(Wq, dtype=np.float32)
    Wk = np.asarray(Wk, dtype=np.float32)
    Wv = np.asarray(Wv, dtype=np.float32)
    Wo = np.asarray(Wo, dtype=np.float32)
    bo = np.asarray(bo, dtype=np.float32)

    if not _alibi_is_structured(alibi_bias):
        return _reference_fallback(query, key, value, alibi_bias, Wq, Wk, Wv, Wo, bo)

    from concourse import bass_utils

    bands, core_heads = _plan_shards(alibi_bias, query.shape[1])
    if bands not in _NC_CACHE:
        _NC_CACHE[bands] = build_nc(bands)
    nc = _NC_CACHE[bands]

    in_maps = _host_prep(query, key, value, alibi_bias, Wq, Wk, Wv, Wo, core_heads)
    res = bass_utils.run_bass_kernel_spmd(nc, in_maps, core_ids=list(range(NCORES)))
    b_sz, t_sz, d_sz = query.shape
    per_b = NCORES // b_sz
    outp = np.zeros((b_sz, t_sz, d_sz), dtype=np.float64)
    for c in range(NCORES):
        outp[c // per_b] += res.results[c]["out"]
    return (outp + bo).astype(np.float32)


if __name__ == "__main__":
    pass


# revision 20
# speedup vs baseline: 1.1184x; 1.0695x over previous
"""ALiBi multi-head causal attention on 8 TRN2 NeuronCores.

Sharding: each core owns ONE batch (b = core//4) and FOUR heads, one from
each "band group". ALiBi weights decay as e^(-slope*dist), so head h only
needs keys within dist <= d_h = 34/slope_h of the query (dropped tail is
< e^-22 relative — far below the error budget). Heads are sorted by band
width and grouped in fours; group s's band (max over the group) is baked
into the one SPMD graph as slot s, and core c takes member c%4 of each
group. This balances the sparse-attention work across cores while keeping
a single NEFF. Bands are computed from the actual slopes in the input, so
a non-decaying alibi (e.g. zeros) degrades to full causal attention, never
to a wrong answer.

Device algorithm per head slot:
  - Q^T, K^T feature-on-partition from the projections; one extra
    contraction row carries -slope*q (bf16 — any per-q shift cancels in
    softmax, it only needs to keep exp in range), so the scores matmul
    S^T[k, q] lands pre-shifted.
  - exp(S^T + slope*k - 8) on ScalarE with per-partition f32 bias; softmax
    over k (the partition axis) needs no reduction: the denominator is the
    ones-column of V through the same PV matmul.
  - Causality: future tiles are never emitted; diagonal tiles get a
    triangular affine_select zero-fill after exp. Band: tiles left of
    q - d_slot are never emitted.
  - PV accumulates O^T (+denominator row) in PSUM over k-chunks; normalize
    via reciprocal + DRAM-bounce partition broadcast + multiply.
  - Output projection accumulates both 128-feature halves into [t, d] and
    streams out. Host sums the 4 per-core partials of each batch, adds bo.
"""

import sys

sys.path.insert(0, "/opt/trn_rl_repo")

import math

import numpy as np

B, T, D, H = 2, 2048, 1024, 16
DK = 64
NCORES = 8
HPS = 4  # head slots per core
FS = HPS * DK  # feature slice per core = 256

TQ = 512  # q-group width (one fp32 psum bank)
TK = 128  # k-chunk width (partition dim)
DC = 128  # projection contraction chunk
MARGIN = 34.0  # band cut: contributions with slope*dist >= MARGIN dropped

_NC_CACHE = {}


def build_nc(bands, t_sz=T, d_sz=D):
    import concourse.bass as bass
    import concourse.mybir as mybir
    import concourse.tile as tile
    from concourse import bacc

    fp32 = mybir.dt.float32
    bf16 = mybir.dt.bfloat16
    EXP = mybir.ActivationFunctionType.Exp

    n_dc = d_sz // DC
    n_kc = t_sz // TK
    n_qg = t_sz // TQ
    n_tc = t_sz // TK

    nc = bacc.Bacc("TRN2", target_bir_lowering=False, debug=False)

    qT = nc.declare_dram_parameter("qT", [d_sz, t_sz], bf16, isOutput=False)
    kT = nc.declare_dram_parameter("kT", [d_sz, t_sz], bf16, isOutput=False)
    vT = nc.declare_dram_parameter("vT", [d_sz, t_sz], bf16, isOutput=False)
    wq = nc.declare_dram_parameter("wq", [n_dc, DC, FS], bf16, isOutput=False)
    wk = nc.declare_dram_parameter("wk", [n_dc, DC, FS], bf16, isOutput=False)
    wv = nc.declare_dram_parameter("wv", [n_dc, DC, FS], bf16, isOutput=False)
    wo = nc.declare_dram_parameter("wo", [2, DC, d_sz], bf16, isOutput=False)
    qaug = nc.declare_dram_parameter("qaug", [HPS, t_sz], bf16, isOutput=False)
    ident = nc.declare_dram_parameter("ident", [128, 128], bf16, isOutput=False)
    ebias = nc.declare_dram_parameter("ebias", [TK, HPS, n_kc], fp32, isOutput=False)
    out = nc.declare_dram_parameter("out", [t_sz, d_sz], fp32, isOutput=True)

    # tiling plan per (slot, q-half): (j, g, s0, start, stop) per score tile.
    # q-halves let the [65, t_sz/2] O^T accumulator double-buffer in PSUM so
    # the next half's PV work overlaps this half's normalization tail.
    n_sp = 2 if n_qg >= 2 else 1  # q-half splits
    n_gh = n_qg // n_sp  # q-groups per half
    plans = []
    for s in range(HPS):
        d = int(bands[s])
        half_plans = []
        for qh in range(n_sp):
            tiles = []
            first_j = {}
            last_j = {}
            for j in range(n_kc):
                g0 = (j * TK) // TQ
                gmax = min(n_qg - 1, (j * TK + TK - 1 + d) // TQ)
                for g in range(max(g0, qh * n_gh), min(gmax, (qh + 1) * n_gh - 1) + 1):
                    if g not in first_j:
                        first_j[g] = j
                    last_j[g] = j
                    tiles.append((j, g))
            plan = []
            for j, g in tiles:
                s0 = j * TK - g * TQ if j * TK > g * TQ else 0
                plan.append((j, g, s0, j == first_j[g], j == last_j[g]))
            half_plans.append(plan)
        plans.append(half_plans)

    with tile.TileContext(nc) as tc:
        with (
            tc.tile_pool(name="const", bufs=1) as cpool,
            tc.tile_pool(name="proj", bufs=2) as projpool,
            tc.tile_pool(name="inp", bufs=8) as inpool,
            tc.tile_pool(name="work", bufs=4) as wpool,
            tc.tile_pool(name="ps", bufs=2, space="PSUM") as ps,
            tc.tile_pool(name="otps", bufs=1, space="PSUM") as otps,
            tc.tile_pool(name="dram", bufs=2, space="DRAM") as dpool,
        ):
            # ---- constants ----
            wq_sb = cpool.tile([DC, n_dc, FS], bf16)
            wk_sb = cpool.tile([DC, n_dc, FS], bf16)
            wv_sb = cpool.tile([DC, n_dc, FS], bf16)
            for w_sb, w_dr in ((wq_sb, wq), (wk_sb, wk), (wv_sb, wv)):
                nc.sync.dma_start(w_sb[:], w_dr.rearrange("c p f -> p c f"))
            wo_sb = cpool.tile([DC, 2, d_sz], bf16)
            nc.sync.dma_start(wo_sb[:], wo.rearrange("h p f -> p h f"))
            ebias_sb = cpool.tile([TK, HPS * n_kc], fp32)
            nc.sync.dma_start(ebias_sb[:], ebias.rearrange("p h j -> p (h j)"))
            ident_sb = cpool.tile([128, 128], bf16)
            nc.sync.dma_start(ident_sb[:], ident[:])

            # ---- phase A: projections -> per-slot QT/KT, per-fh VT ----
            qt_t = [
                projpool.tile([DK + 1, t_sz], bf16, tag="qt", name=f"qt{s}", bufs=HPS)
                for s in range(HPS)
            ]
            kt_t = [
                projpool.tile([DK + 1, t_sz], bf16, tag="kt", name=f"kt{s}", bufs=HPS)
                for s in range(HPS)
            ]
            vt_t = [
                projpool.tile([128, t_sz], bf16, tag="vt", name=f"vt{fh}", bufs=2)
                for fh in range(2)
            ]

            for s in range(HPS):
                nc.gpsimd.dma_start(qt_t[s][DK : DK + 1, :], qaug[s : s + 1, :])
                nc.gpsimd.memset(kt_t[s][DK : DK + 1, :], 1.0)

            for w_sb, xdr, kind in ((wq_sb, qT, "q"), (wk_sb, kT, "k"), (wv_sb, vT, "v")):
                for g in range(n_qg):
                    mm_f = [
                        ps.tile([128, TQ], fp32, tag="mm", name=f"mm{kind}{g}_{fh}")
                        for fh in range(2)
                    ]
                    for dc in range(n_dc):
                        xin = inpool.tile([DC, TQ], bf16, tag="xin")
                        nc.sync.dma_start(
                            xin[:],
                            xdr[dc * DC : (dc + 1) * DC, g * TQ : (g + 1) * TQ],
                        )
                        for fh in range(2):
                            nc.tensor.matmul(
                                mm_f[fh][:],
                                w_sb[:, dc, fh * DC : (fh + 1) * DC],
                                xin[:],
                                start=(dc == 0),
                                stop=(dc == n_dc - 1),
                            )
                    gs = slice(g * TQ, (g + 1) * TQ)
                    for fh in range(2):
                        if kind == "q":
                            nc.vector.tensor_copy(qt_t[2 * fh][0:DK, gs], mm_f[fh][0:DK, :])
                            nc.vector.tensor_copy(qt_t[2 * fh + 1][0:DK, gs], mm_f[fh][DK:128, :])
                        elif kind == "k":
                            nc.vector.tensor_copy(kt_t[2 * fh][0:DK, gs], mm_f[fh][0:DK, :])
                            nc.vector.tensor_copy(kt_t[2 * fh + 1][0:DK, gs], mm_f[fh][DK:128, :])
                        else:
                            nc.vector.tensor_copy(vt_t[fh][:, gs], mm_f[fh][:])

            # ---- phase A2: V transpose (PE) -> per-slot V_aug [k, j, DK+1] ----
            va_t = [
                projpool.tile([TK, n_kc, DK + 1], bf16, tag="va", name=f"va{s}", bufs=HPS)
                for s in range(HPS)
            ]
            for s in range(HPS):
                nc.gpsimd.memset(va_t[s][:, :, DK : DK + 1], 1.0)
            for fh in range(2):
                for j in range(n_kc):
                    vtr = ps.tile([128, 128], bf16, tag="st", name=f"vtr{fh}_{j}")
                    nc.tensor.transpose(
                        vtr[:], vt_t[fh][:, j * TK : (j + 1) * TK], ident_sb[:]
                    )
                    nc.vector.tensor_copy(va_t[2 * fh][:, j, 0:DK], vtr[:, 0:DK])
                    nc.vector.tensor_copy(va_t[2 * fh + 1][:, j, 0:DK], vtr[:, DK:128])

            # ---- phase B: attention per slot ----
            otn_t = [
                projpool.tile([128, t_sz], bf16, tag="otn", name=f"otn{fh}", bufs=2)
                for fh in range(2)
            ]
            for s in range(HPS):
              for qh in range(n_sp):
                hbase = qh * (t_sz // n_sp)
                ot = otps.tile([DK + 1, t_sz // n_sp], fp32, tag="ot", name=f"ot{s}_{qh}", bufs=2)
                for j, g, s0, first, last in plans[s][qh]:
                    st = ps.tile([128, TQ], fp32, tag="st", name=f"st{s}_{j}_{g}")
                    nc.tensor.matmul(
                        st[:, s0:TQ],
                        kt_t[s][:, j * TK : (j + 1) * TK],
                        qt_t[s][:, g * TQ + s0 : (g + 1) * TQ],
                        start=True,
                        stop=True,
                    )
                    pt = wpool.tile([128, TQ], bf16, tag="pt", name=f"pt{s}_{j}_{g}")
                    nc.scalar.activation(
                        pt[:, s0:TQ],
                        st[:, s0:TQ],
                        EXP,
                        bias=ebias_sb[:, s * n_kc + j : s * n_kc + j + 1],
                        scale=1.0,
                    )
                    if j * TK >= g * TQ:
                        nc.gpsimd.affine_select(
                            out=pt[:, s0 : s0 + TK],
                            in_=pt[:, s0 : s0 + TK],
                            compare_op=mybir.AluOpType.is_ge,
                            fill=0.0,
                            base=0,
                            pattern=[[1, TK]],
                            channel_multiplier=-1,
                        )
                    nc.tensor.matmul(
                        ot[:, g * TQ + s0 - hbase : (g + 1) * TQ - hbase],
                        va_t[s][:, j, :],
                        pt[:, s0:TQ],
                        start=first,
                        stop=last,
                    )
                # normalize: recip of denominator row, broadcast, multiply
                th = t_sz // n_sp
                recip = wpool.tile([1, th], fp32, tag="recip", bufs=2, name=f"recip{s}_{qh}")
                nc.vector.reciprocal(recip[0:1, :], ot[DK : DK + 1, :])
                rb_dram = dpool.tile([1, th], fp32, name=f"rbd{s}_{qh}")
                nc.sync.dma_start(rb_dram[:], recip[0:1, :])
                rb_ap = rb_dram[:]
                rb_bcast = type(rb_ap)(rb_ap.tensor, rb_ap.offset, [[0, DK], [1, th]])
                rbc = wpool.tile([DK, th], fp32, tag="rbc", bufs=2, name=f"rbc{s}_{qh}")
                nc.sync.dma_start(rbc[:], rb_bcast)
                nc.vector.tensor_tensor(
                    out=otn_t[s // 2][(s % 2) * DK : (s % 2) * DK + DK, hbase : hbase + th],
                    in0=ot[0:DK, :],
                    in1=rbc[:],
                    op=mybir.AluOpType.mult,
                )

            # ---- phase C: output projection ----
            for tci in range(n_tc):
                for dh in range(d_sz // TQ):
                    og = ps.tile([128, TQ], fp32, tag="mm", name=f"og{tci}_{dh}")
                    for fh in range(2):
                        nc.tensor.matmul(
                            og[:],
                            otn_t[fh][:, tci * TK : (tci + 1) * TK],
                            wo_sb[:, fh, dh * TQ : (dh + 1) * TQ],
                            start=(fh == 0),
                            stop=(fh == 1),
                        )
                    ob = wpool.tile([128, TQ], fp32, tag="ob", name=f"ob{tci}_{dh}")
                    if dh % 2 == 0:
                        nc.vector.tensor_copy(ob[:], og[:])
                    else:
                        nc.scalar.copy(ob[:], og[:])
                    nc.sync.dma_start(
                        out[tci * TK : (tci + 1) * TK, dh * TQ : (dh + 1) * TQ],
                        ob[:],
                    )

    nc.compile()
    return nc


def _plan_shards(alibi_bias, t_sz):
    """Head->slot assignment and per-slot bands from the actual slopes."""
    slopes = (-alibi_bias[:, 1, 0]).astype(np.float64)
    d = np.where(slopes > 0, np.ceil(MARGIN / np.maximum(slopes, 1e-30)), t_sz)
    d = np.minimum(d, t_sz).astype(np.int64)
    order = np.argsort(-d, kind="stable")  # widest band first
    groups = [order[4 * s : 4 * s + 4] for s in range(HPS)]
    bands = tuple(int(d[g].max()) for g in groups)
    core_heads = [[int(groups[s][c % 4]) for s in range(HPS)] for c in range(NCORES)]
    return bands, core_heads


def _host_prep(query, key, value, alibi_bias, Wq, Wk, Wv, Wo, core_heads):
    import ml_dtypes

    bf16 = ml_dtypes.bfloat16
    b_sz, t_sz, d_sz = query.shape

    slopes = (-alibi_bias[:, 1, 0]).astype(np.float32)

    identity = np.eye(128).astype(bf16)
    qTh = [np.ascontiguousarray(query[b].T).astype(bf16) for b in range(b_sz)]
    kTh = [np.ascontiguousarray(key[b].T).astype(bf16) for b in range(b_sz)]
    vTh = [np.ascontiguousarray(value[b].T).astype(bf16) for b in range(b_sz)]

    scale = 1.0 / math.sqrt(DK)
    n_dc = d_sz // DC
    n_kc = t_sz // TK
    qpos = np.arange(t_sz, dtype=np.float32)
    p = np.arange(TK, dtype=np.float32)
    jj = np.arange(n_kc, dtype=np.float32)
    kpos = jj[None, :] * TK + p[:, None]  # [TK, n_kc]

    per_b = NCORES // b_sz
    in_maps = []
    for c in range(NCORES):
        b = c // per_b
        heads = core_heads[c]
        rows = np.concatenate([np.arange(h * DK, (h + 1) * DK) for h in heads])
        hsl = slopes[heads]  # [HPS]
        wq_c = np.ascontiguousarray((Wq[rows] * scale).T.reshape(n_dc, DC, FS)).astype(bf16)
        wk_c = np.ascontiguousarray(Wk[rows].T.reshape(n_dc, DC, FS)).astype(bf16)
        wv_c = np.ascontiguousarray(Wv[rows].T.reshape(n_dc, DC, FS)).astype(bf16)
        wo_c = np.ascontiguousarray(Wo[:, rows].T.reshape(2, DC, d_sz)).astype(bf16)
        qaug_c = (-hsl[:, None] * qpos[None, :]).astype(bf16)
        ebias_c = np.ascontiguousarray(
            (hsl[None, :, None] * kpos[:, None, :] - 8.0).astype(np.float32)
        )
        in_maps.append(
            {
                "qT": qTh[b],
                "kT": kTh[b],
                "vT": vTh[b],
                "wq": wq_c,
                "wk": wk_c,
                "wv": wv_c,
                "wo": wo_c,
                "qaug": qaug_c,
                "ebias": ebias_c,
                "ident": identity,
            }
        )
    return in_maps


def _alibi_is_structured(alibi_bias):
    """Check bias[h,i,j] == slope_h*(j-i) on a sample grid."""
    hgrid = np.arange(alibi_bias.shape[0])
    igrid = np.linspace(0, alibi_bias.shape[1] - 1, 37).astype(np.int64)
    jgrid = np.linspace(0, alibi_bias.shape[2] - 1, 41).astype(np.int64)
    slopes = (-alibi_bias[:, 1, 0]).astype(np.float32)
    sample = alibi_bias[np.ix_(hgrid, igrid, jgrid)].astype(np.float32)
    dist = jgrid[None, :].astype(np.float32) - igrid[:, None].astype(np.float32)
    ref = slopes[:, None, None] * dist[None]
    return np.allclose(sample, ref, rtol=1e-5, atol=1e-6)


def _reference_fallback(query, key, value, alibi_bias, Wq, Wk, Wv, Wo, bo):
    b_sz, t_sz, d_sz = query.shape
    n_heads = alibi_bias.shape[0]
    dk = d_sz // n_heads
    q64, k64, v64 = (x.astype(np.float64) for x in (query, key, value))
    Q = (q64 @ Wq.T.astype(np.float64)).reshape(b_sz, t_sz, n_heads, dk)
    K = (k64 @ Wk.T.astype(np.float64)).reshape(b_sz, t_sz, n_heads, dk)
    V = (v64 @ Wv.T.astype(np.float64)).reshape(b_sz, t_sz, n_heads, dk)
    out = np.zeros((b_sz, t_sz, d_sz), dtype=np.float64)
    causal = np.triu(np.ones((t_sz, t_sz), dtype=bool), 1)
    for b in range(b_sz):
        for h in range(n_heads):
            s = (Q[b, :, h] @ K[b, :, h].T) / math.sqrt(dk) + alibi_bias[h]
            s = np.where(causal, -np.inf, s)
            s -= s.max(axis=-1, keepdims=True)
            pr = np.exp(s)
            pr /= pr.sum(axis=-1, keepdims=True)
            out[b, :, h * dk : (h + 1) * dk] = pr @ V[b, :, h]
    return (
        (out.reshape(b_sz * t_sz, d_sz) @ Wo.T.astype(np.float64) + bo)
        .reshape(b_sz, t_sz, d_sz)
        .astype(np.float32)
    )


def kernel(query, key, value, alibi_bias, Wq, Wk, Wv, Wo, bo):
    query = np.asarray(query, dtype=np.float32)
    key = np.asarray(key, dtype=np.float32)
    value = np.asarray(value, dtype=np.float32)
    alibi_bias = np.asarray(alibi_bias, dtype=np.float32)
    Wq = np.asarray(Wq, dtype=np.float32)
    Wk = np.asarray(Wk, dtype=np.float32)
    Wv = np.asarray(Wv, dtype=np.float32)
    Wo = np.asarray(Wo, dtype=np.float32)
    bo = np.asarray(bo, dtype=np.float32)

    if not _alibi_is_structured(alibi_bias):
        return _reference_fallback(query, key, value, alibi_bias, Wq, Wk, Wv, Wo, bo)

    from concourse import bass_utils

    bands, core_heads = _plan_shards(alibi_bias, query.shape[1])
    if bands not in _NC_CACHE:
        _NC_CACHE[bands] = build_nc(bands)
    nc = _NC_CACHE[bands]

    in_maps = _host_prep(query, key, value, alibi_bias, Wq, Wk, Wv, Wo, core_heads)
    res = bass_utils.run_bass_kernel_spmd(nc, in_maps, core_ids=list(range(NCORES)))
    b_sz, t_sz, d_sz = query.shape
    per_b = NCORES // b_sz
    outp = np.zeros((b_sz, t_sz, d_sz), dtype=np.float64)
    for c in range(NCORES):
        outp[c // per_b] += res.results[c]["out"]
    return (outp + bo).astype(np.float32)


if __name__ == "__main__":
    pass
